# revision 6
# baseline (speedup 1.0000x reference)
"""CenterLoss update kernel for Trainium2, 8-core SPMD.

Reference computation (N=16384 samples, C=10000 classes, D=128 dims):
    embeded_labels = labels @ center          # [N,D] gather via one-hot
    diff = embeded_labels - embeded_preds
    grad = (labels.T @ diff) / (counts + 1)   # counts = labels.T @ ones
    out  = center - 0.5 * grad

Because each row of ``labels`` is one-hot, ``labels.T @ labels == diag(counts)``,
so the whole thing collapses to a single pass over ``labels``:

    S      = labels.T @ embeded_preds         # [C,D] per-class sum of preds
    counts = column sums of labels            # [C]
    out    = beta * center + gamma * S
             beta  = 1 - 0.5*counts/(counts+1)
             gamma = 0.5/(counts+1)

The 655MB ``labels`` tensor is streamed through the PE exactly once as the
moving matmul operand (computing S.T = preds.T @ labels tile by tile) in a
single fp32r pass (~1e-4 relative error, far inside the 2e-2 gate), with
per-partition partial counts accumulated on the vector engine and reduced by
a ones matmul at each group boundary.  Work is data-parallel over N across 8
cores.  Classes are processed in 10 column groups of 1000; each group's
partial (S.T ; counts) block is cast to fp16 and staged to DRAM, and three
ReduceScatters (groups 0-3, 4-7, 8-9) run *during* the label streaming so
only the last small collective is exposed at the tail.  Each ReduceScatter
hands core i columns [i*125, (i+1)*125) of every group in its batch, so the
final elementwise update is local; the host reassembles the group-interleaved
class order.
"""

import numpy as np

N, C, D = 16384, 10000, 128
NCORES = 8
NS = N // NCORES        # 2048 rows per core
CS = C // NCORES        # 1250 classes per core
LR = 0.5
P = 128
KT = NS // P            # 16 k-tiles over this core's rows
NG = 10                 # class-column groups
GW = C // NG            # 1000 columns per group
SH = GW // NCORES       # 125 columns per core per group
# (start group, group count) per ReduceScatter batch; padded row count chosen
# so each per-rank shard (rows * nb*SH * 2B) is a 32-byte multiple.
BATCHES = [(0, 4, 132), (4, 4, 132), (8, 2, 136)]


def _chunks(width, step=512):
    out = []
    c0 = 0
    while c0 < width:
        out.append((c0, min(step, width - c0)))
        c0 += step
    return out


def build_program(ns=NS, c=C, d=D, ncores=NCORES):
    """Build the SPMD Bass program (identical on every core)."""
    import concourse.bacc as bacc
    import concourse.mybir as mybir
    import concourse.tile as tile
    from concourse.masks import make_identity

    f32 = mybir.dt.float32
    f32r = mybir.dt.float32r
    f16 = mybir.dt.float16
    mult = mybir.AluOpType.mult
    add = mybir.AluOpType.add

    assert ns % P == 0 and c % NG == 0 and GW % ncores == 0

    nc = bacc.Bacc(
        "TRN2",
        target_bir_lowering=False,
        debug=False,
        num_devices=ncores,
    )

    # preds/labels are declared float32r (same bits as the host's fp32) so
    # plain HWDGE DMAs can feed fp32r matmuls at full speed (1 cycle/row vs 4
    # for fp32); skipping the true mantissa rounding costs ~1e-4 relative
    # error, far inside the 2e-2 gate.
    preds = nc.dram_tensor("preds", [ns, d], f32r, kind="ExternalInput").ap()
    labels = nc.dram_tensor("labels", [ns, c], f32r, kind="ExternalInput").ap()
    # center rows arrive pre-permuted to this core's (group, col) order.
    center = nc.dram_tensor("center", [CS, d], f32, kind="ExternalInput").ap()
    out = nc.dram_tensor("out", [CS, d], f32, kind="ExternalOutput").ap()

    # phase-3 tiles: (batch, col offset in batch, width, out row start)
    p3_chunks = []
    for b, (g0, nb, _rp) in enumerate(BATCHES):
        wb = nb * SH
        o = 0
        while o < wb:
            w = min(P, wb - o)
            p3_chunks.append((b, o, w, g0 * SH + o))
            o += w

    batch_of = {}
    for b, (g0, nb, _rp) in enumerate(BATCHES):
        for g in range(g0, g0 + nb):
            batch_of[g] = (b, g - g0)

    with tile.TileContext(nc) as tc:
        with (
            tc.tile_pool(name="const", bufs=1) as const_pool,
            tc.tile_pool(name="dram", bufs=1, space="DRAM") as dram_pool,
            tc.tile_pool(name="lab", bufs=8) as lab_pool,
            tc.tile_pool(name="cnts", bufs=2) as cnts_pool,
            tc.tile_pool(name="stage", bufs=2) as stage_pool,
            tc.tile_pool(name="psum", bufs=1, space="PSUM") as psum,
            tc.tile_pool(name="p3", bufs=2) as p3_pool,
        ):
            identity_h = const_pool.tile([P, P], f16, name="identity_h")
            make_identity(nc, identity_h[:])
            ones_col = const_pool.tile([P, 1], f32, name="ones_col")
            nc.vector.memset(ones_col[:], 1.0)

            # preds for this core, as KT stationary [K=128, M=d] tiles
            # (single reduced-precision fp32r PE pass).
            preds_sb = const_pool.tile([P, KT * d], f32r, name="preds_sb")
            for t in range(KT):
                nc.sync.dma_start(
                    out=preds_sb[:, t * d:(t + 1) * d],
                    in_=preds[t * P:(t + 1) * P, :],
                )

            # staging + reduce buffers per ReduceScatter batch
            partials, reds = [], []
            for b, (g0, nb, rp) in enumerate(BATCHES):
                wb = nb * SH
                partial = dram_pool.tile(
                    [ncores, rp, wb], f16, name=f"partial_{b}"
                )
                red = dram_pool.tile([rp, wb], f16, name=f"red_{b}")
                partials.append(partial)
                reds.append(red)

            def stage_group(g, st_psum, counts_g):
                b, goff = batch_of[g]
                cnt_psum = psum.tile(
                    [1, GW], f32, name=f"cnt_psum_{g}", tag="cntp", space="PSUM"
                )
                for c0, w in _chunks(GW):
                    # plain fp32 matmul (counts must not be rounded); only
                    # ~3us of PE time per group
                    nc.tensor.matmul(
                        out=cnt_psum[0:1, c0:c0 + w],
                        lhsT=ones_col[:],
                        rhs=counts_g[:, c0:c0 + w],
                        start=True,
                        stop=True,
                    )
                st_stage = stage_pool.tile(
                    [P, GW], f16, name=f"st_stage_{g}", tag="st_stage"
                )
                nc.scalar.copy(out=st_stage[:], in_=st_psum[:])
                cnt_stage = stage_pool.tile(
                    [1, GW], f16, name=f"cnt_stage_{g}", tag="cnt_stage"
                )
                nc.scalar.copy(out=cnt_stage[:], in_=cnt_psum[:])
                off = goff * SH
                for i in range(ncores):
                    nc.sync.dma_start(
                        out=partials[b][i, 0:d, off:off + SH],
                        in_=st_stage[:, i * SH:(i + 1) * SH],
                    )
                    nc.sync.dma_start(
                        out=partials[b][i, d:d + 1, off:off + SH],
                        in_=cnt_stage[0:1, i * SH:(i + 1) * SH],
                    )

            def reduce_batch(b):
                nc.gpsimd.collective_compute(
                    "ReduceScatter",
                    mybir.AluOpType.add,
                    replica_groups=[list(range(ncores))],
                    ins=[partials[b][:].opt()],
                    outs=[reds[b][:].opt()],
                )

            def phase3_batch(b):
                g0, nb, _rp = BATCHES[b]
                wb = nb * SH
                st_sh = p3_pool.tile([P, wb], f16, name=f"st_sh_{b}", tag="st_sh")
                nc.sync.dma_start(out=st_sh[:, 0:wb], in_=reds[b][0:d, :])
                cnt_row = p3_pool.tile(
                    [1, wb], f16, name=f"cnt_row_{b}", tag="cnt_row"
                )
                nc.sync.dma_start(out=cnt_row[:, 0:wb], in_=reds[b][d:d + 1, :])
                for (bb, o, w, r0) in p3_chunks:
                    if bb != b:
                        continue
                    ctr_t = p3_pool.tile([P, d], f32, name=f"ctr_{b}_{o}", tag="ctr")
                    nc.sync.dma_start(out=ctr_t[0:w, :], in_=center[r0:r0 + w, :])
                    trp = psum.tile(
                        [P, d], f16, name=f"trp_{b}_{o}", tag="trp", space="PSUM"
                    )
                    nc.tensor.transpose(
                        out=trp[0:w, 0:d],
                        in_=st_sh[:, o:o + w],
                        identity=identity_h[:, 0:d],
                    )
                    cntc = psum.tile(
                        [P, 1], f16, name=f"cntc_{b}_{o}", tag="cntc", space="PSUM"
                    )
                    nc.tensor.transpose(
                        out=cntc[0:w, 0:1],
                        in_=cnt_row[0:1, o:o + w],
                        identity=identity_h[0:1, 0:1],
                    )
                    den = p3_pool.tile([P, 1], f32, name=f"den_{b}_{o}", tag="den")
                    nc.vector.tensor_scalar_add(
                        out=den[0:w, :], in0=cntc[0:w, :], scalar1=1.0
                    )
                    rec = p3_pool.tile([P, 1], f32, name=f"rec_{b}_{o}", tag="rec")
                    nc.vector.reciprocal(out=rec[0:w, :], in_=den[0:w, :])
                    gam = p3_pool.tile([P, 1], f32, name=f"gam_{b}_{o}", tag="gam")
                    nc.vector.tensor_scalar_mul(
                        out=gam[0:w, :], in0=rec[0:w, :], scalar1=0.5
                    )
                    bet = p3_pool.tile([P, 1], f32, name=f"bet_{b}_{o}", tag="bet")
                    nc.vector.tensor_tensor(
                        out=bet[0:w, :], in0=cntc[0:w, :], in1=rec[0:w, :], op=mult
                    )
                    nc.vector.tensor_scalar(
                        out=bet[0:w, :], in0=bet[0:w, :],
                        scalar1=-0.5, scalar2=1.0, op0=mult, op1=add,
                    )
                    o1 = p3_pool.tile([P, d], f32, name=f"o1_{b}_{o}", tag="o1")
                    nc.vector.tensor_scalar_mul(
                        out=o1[0:w, :], in0=ctr_t[0:w, :], scalar1=bet[0:w, :]
                    )
                    ou = p3_pool.tile([P, d], f32, name=f"ou_{b}_{o}", tag="ou")
                    nc.vector.scalar_tensor_tensor(
                        out=ou[0:w, :], in0=trp[0:w, 0:d], scalar=gam[0:w, :],
                        in1=o1[0:w, :], op0=mult, op1=add,
                    )
                    nc.sync.dma_start(
                        out=out[r0:r0 + w, :], in_=ou[0:w, 0:d]
                    )

            # ---------------- phase 1: stream labels ----------------
            for g in range(NG):
                st_psum = psum.tile(
                    [d, GW], f32, name=f"st_psum_{g}", tag="st", bufs=2,
                    space="PSUM",
                )
                counts_g = cnts_pool.tile(
                    [P, GW], f32, name=f"counts_{g}", tag="cnt_sb"
                )
                for t in range(KT):
                    lab_t = lab_pool.tile(
                        [P, GW], f32r, name=f"lab_{g}_{t}", tag="lab"
                    )
                    nc.sync.dma_start(
                        out=lab_t[:],
                        in_=labels[t * P:(t + 1) * P, g * GW:(g + 1) * GW],
                    )
                    for c0, w in _chunks(GW):
                        nc.tensor.matmul(
                            out=st_psum[:, c0:c0 + w],
                            lhsT=preds_sb[:, t * d:(t + 1) * d],
                            rhs=lab_t[:, c0:c0 + w],
                            start=(t == 0),
                            stop=(t == KT - 1),
                        )
                    if t == 0:
                        nc.vector.tensor_copy(
                            out=counts_g[:], in_=lab_t[:].bitcast(f32)
                        )
                    else:
                        nc.vector.tensor_add(
                            out=counts_g[:],
                            in0=counts_g[:],
                            in1=lab_t[:].bitcast(f32),
                        )
                stage_group(g, st_psum, counts_g)
                if g == 3:
                    reduce_batch(0)
                elif g == 5:
                    phase3_batch(0)
                elif g == 7:
                    reduce_batch(1)
                elif g == 9:
                    reduce_batch(2)

            # ---------------- tail: remaining updates ----------------
            phase3_batch(1)
            phase3_batch(2)

    nc.compile()
    return nc


_PROGRAM = None
LAST_RESULTS = None  # BassKernelResults from the most recent run (for test.py)


def _get_program():
    global _PROGRAM
    if _PROGRAM is None:
        _PROGRAM = build_program()
    return _PROGRAM


def kernel(embeded_preds, labels, center):
    from concourse.bass_utils import run_bass_kernel_spmd

    global LAST_RESULTS
    preds = np.ascontiguousarray(np.asarray(embeded_preds, dtype=np.float32))
    lab = np.ascontiguousarray(np.asarray(labels, dtype=np.float32))
    ctr = np.ascontiguousarray(np.asarray(center, dtype=np.float32))
    assert preds.shape == (N, D) and lab.shape == (N, C) and ctr.shape == (C, D)

    nc = _get_program()
    # core i's classes are {g*GW + i*SH + k}: feed it the matching center rows
    ctr_g = ctr.reshape(NG, NCORES, SH, D)
    in_maps = [
        {
            "preds": preds[i * NS:(i + 1) * NS],
            "labels": lab[i * NS:(i + 1) * NS],
            "center": np.ascontiguousarray(ctr_g[:, i].reshape(CS, D)),
        }
        for i in range(NCORES)
    ]
    res = run_bass_kernel_spmd(nc, in_maps, core_ids=list(range(NCORES)))
    LAST_RESULTS = res
    outs = np.stack([res.results[i]["out"] for i in range(NCORES)], axis=0)
    return np.ascontiguousarray(
        outs.reshape(NCORES, NG, SH, D).transpose(1, 0, 2, 3).reshape(C, D)
    )


# revision 12
# speedup vs baseline: 1.0194x; 1.0194x over previous
"""CenterLoss update kernel for Trainium2, 8-core SPMD.

Reference computation (N=16384 samples, C=10000 classes, D=128 dims):
    embeded_labels = labels @ center          # [N,D] gather via one-hot
    diff = embeded_labels - embeded_preds
    grad = (labels.T @ diff) / (counts + 1)   # counts = labels.T @ ones
    out  = center - 0.5 * grad

Because each row of ``labels`` is one-hot, ``labels.T @ labels == diag(counts)``,
so the whole thing collapses to a single pass over ``labels``:

    S      = labels.T @ embeded_preds         # [C,D] per-class sum of preds
    counts = column sums of labels            # [C]
    out    = beta * center + gamma * S
             beta  = 1 - 0.5*counts/(counts+1)
             gamma = 0.5/(counts+1)

The 655MB ``labels`` tensor is streamed through the PE exactly once as the
moving matmul operand (computing S.T = preds.T @ labels tile by tile) in a
single fp32r pass (~1e-4 relative error, far inside the 2e-2 gate), with
per-partition partial counts accumulated on the vector engine and reduced by
a ones matmul at each group boundary.  Work is data-parallel over N across 8
cores.  Classes are processed in 10 column groups of 1000; each group's
partial (S.T ; counts) block is cast to fp16 and staged to DRAM, and three
ReduceScatters (groups 0-3, 4-7, 8-9) run *during* the label streaming so
only the last small collective is exposed at the tail.  Each ReduceScatter
hands core i columns [i*125, (i+1)*125) of every group in its batch, so the
final elementwise update is local; the host reassembles the group-interleaved
class order.
"""

import numpy as np

N, C, D = 16384, 10000, 128
NCORES = 8
NS = N // NCORES        # 2048 rows per core
CS = C // NCORES        # 1250 classes per core
LR = 0.5
P = 128
KT = NS // P            # 16 k-tiles over this core's rows
NG = 10                 # class-column groups
GW = C // NG            # 1000 columns per group
SH = GW // NCORES       # 125 columns per core per group
# (start group, group count, padded row count) per ReduceScatter batch; the
# pad makes each per-rank shard (rows * nb*SH * 2B) a 32-byte multiple.
# Collectives have a ~30us fixed cost, so use just two: one mid-stream
# (hidden under label streaming) and one at the tail.
BATCHES = [(0, 6, 136), (6, 4, 132)]


def _chunks(width, step=512):
    out = []
    c0 = 0
    while c0 < width:
        out.append((c0, min(step, width - c0)))
        c0 += step
    return out


def build_program(ns=NS, c=C, d=D, ncores=NCORES):
    """Build the SPMD Bass program (identical on every core)."""
    import concourse.bacc as bacc
    import concourse.mybir as mybir
    import concourse.tile as tile
    from concourse.masks import make_identity

    f32 = mybir.dt.float32
    f32r = mybir.dt.float32r
    f16 = mybir.dt.float16
    mult = mybir.AluOpType.mult
    add = mybir.AluOpType.add

    assert ns % P == 0 and c % NG == 0 and GW % ncores == 0

    nc = bacc.Bacc(
        "TRN2",
        target_bir_lowering=False,
        debug=False,
        num_devices=ncores,
    )

    # preds/labels are declared float32r (same bits as the host's fp32) so
    # plain HWDGE DMAs can feed fp32r matmuls at full speed (1 cycle/row vs 4
    # for fp32); skipping the true mantissa rounding costs ~1e-4 relative
    # error, far inside the 2e-2 gate.
    preds = nc.dram_tensor("preds", [ns, d], f32r, kind="ExternalInput").ap()
    labels = nc.dram_tensor("labels", [ns, c], f32r, kind="ExternalInput").ap()
    # center rows arrive pre-permuted to this core's (group, col) order.
    center = nc.dram_tensor("center", [CS, d], f32, kind="ExternalInput").ap()
    out = nc.dram_tensor("out", [CS, d], f32, kind="ExternalOutput").ap()

    # phase-3 tiles: (batch, col offset in batch, width, out row start)
    p3_chunks = []
    for b, (g0, nb, _rp) in enumerate(BATCHES):
        wb = nb * SH
        o = 0
        while o < wb:
            w = min(P, wb - o)
            p3_chunks.append((b, o, w, g0 * SH + o))
            o += w

    batch_of = {}
    for b, (g0, nb, _rp) in enumerate(BATCHES):
        for g in range(g0, g0 + nb):
            batch_of[g] = (b, g - g0)

    with tile.TileContext(nc) as tc:
        with (
            tc.tile_pool(name="const", bufs=1) as const_pool,
            tc.tile_pool(name="dram", bufs=1, space="DRAM") as dram_pool,
            tc.tile_pool(name="lab", bufs=8) as lab_pool,
            tc.tile_pool(name="cnts", bufs=2) as cnts_pool,
            tc.tile_pool(name="stage", bufs=2) as stage_pool,
            tc.tile_pool(name="psum", bufs=1, space="PSUM") as psum,
            tc.tile_pool(name="p3", bufs=2) as p3_pool,
        ):
            identity_h = const_pool.tile([P, P], f16, name="identity_h")
            make_identity(nc, identity_h[:])
            ones_col = const_pool.tile([P, 1], f32, name="ones_col")
            nc.vector.memset(ones_col[:], 1.0)

            # preds for this core, as KT stationary [K=128, M=d] tiles
            # (single reduced-precision fp32r PE pass).
            preds_sb = const_pool.tile([P, KT * d], f32r, name="preds_sb")
            for t in range(KT):
                nc.sync.dma_start(
                    out=preds_sb[:, t * d:(t + 1) * d],
                    in_=preds[t * P:(t + 1) * P, :],
                )

            # staging + reduce buffers per ReduceScatter batch
            partials, reds = [], []
            for b, (g0, nb, rp) in enumerate(BATCHES):
                wb = nb * SH
                partial = dram_pool.tile(
                    [ncores, rp, wb], f16, name=f"partial_{b}"
                )
                red = dram_pool.tile([rp, wb], f16, name=f"red_{b}")
                partials.append(partial)
                reds.append(red)

            def stage_group(g, st_psum, counts_a, counts_b):
                b, goff = batch_of[g]
                cnt_psum = psum.tile(
                    [1, GW], f32, name=f"cnt_psum_{g}", tag="cntp", space="PSUM"
                )
                for c0, w in _chunks(GW):
                    # plain fp32 matmuls (counts must not be rounded); only
                    # ~6us of PE time per group.  The two partial-count
                    # accumulators (DVE + GpSimd halves) merge in PSUM.
                    nc.tensor.matmul(
                        out=cnt_psum[0:1, c0:c0 + w],
                        lhsT=ones_col[:],
                        rhs=counts_a[:, c0:c0 + w],
                        start=True,
                        stop=False,
                    )
                    nc.tensor.matmul(
                        out=cnt_psum[0:1, c0:c0 + w],
                        lhsT=ones_col[:],
                        rhs=counts_b[:, c0:c0 + w],
                        start=False,
                        stop=True,
                    )
                st_stage = stage_pool.tile(
                    [P, GW], f16, name=f"st_stage_{g}", tag="st_stage"
                )
                nc.scalar.copy(out=st_stage[:], in_=st_psum[:])
                cnt_stage = stage_pool.tile(
                    [1, GW], f16, name=f"cnt_stage_{g}", tag="cnt_stage"
                )
                nc.scalar.copy(out=cnt_stage[:], in_=cnt_psum[:])
                off = goff * SH
                # issue staging on the ACT ring: the per-rank slices are
                # 250B-chunk strided writes, and on the label ring their
                # descriptor storm head-of-line-blocks the big streaming reads
                for i in range(ncores):
                    nc.scalar.dma_start(
                        out=partials[b][i, 0:d, off:off + SH],
                        in_=st_stage[:, i * SH:(i + 1) * SH],
                    )
                    nc.scalar.dma_start(
                        out=partials[b][i, d:d + 1, off:off + SH],
                        in_=cnt_stage[0:1, i * SH:(i + 1) * SH],
                    )

            def reduce_batch(b):
                nc.gpsimd.collective_compute(
                    "ReduceScatter",
                    mybir.AluOpType.add,
                    replica_groups=[list(range(ncores))],
                    ins=[partials[b][:].opt()],
                    outs=[reds[b][:].opt()],
                )

            def phase3_batch(b):
                g0, nb, _rp = BATCHES[b]
                wb = nb * SH
                st_sh = p3_pool.tile([P, wb], f16, name=f"st_sh_{b}", tag="st_sh")
                nc.gpsimd.dma_start(out=st_sh[:, 0:wb], in_=reds[b][0:d, :])
                cnt_row = p3_pool.tile(
                    [1, wb], f16, name=f"cnt_row_{b}", tag="cnt_row"
                )
                nc.gpsimd.dma_start(out=cnt_row[:, 0:wb], in_=reds[b][d:d + 1, :])
                for (bb, o, w, r0) in p3_chunks:
                    if bb != b:
                        continue
                    ctr_t = p3_pool.tile([P, d], f32, name=f"ctr_{b}_{o}", tag="ctr")
                    nc.gpsimd.dma_start(out=ctr_t[0:w, :], in_=center[r0:r0 + w, :])
                    trp = psum.tile(
                        [P, d], f16, name=f"trp_{b}_{o}", tag="trp", space="PSUM"
                    )
                    nc.tensor.transpose(
                        out=trp[0:w, 0:d],
                        in_=st_sh[:, o:o + w],
                        identity=identity_h[:, 0:d],
                    )
                    cntc = psum.tile(
                        [P, 1], f16, name=f"cntc_{b}_{o}", tag="cntc", space="PSUM"
                    )
                    nc.tensor.transpose(
                        out=cntc[0:w, 0:1],
                        in_=cnt_row[0:1, o:o + w],
                        identity=identity_h[0:1, 0:1],
                    )
                    den = p3_pool.tile([P, 1], f32, name=f"den_{b}_{o}", tag="den")
                    nc.vector.tensor_scalar_add(
                        out=den[0:w, :], in0=cntc[0:w, :], scalar1=1.0
                    )
                    rec = p3_pool.tile([P, 1], f32, name=f"rec_{b}_{o}", tag="rec")
                    nc.vector.reciprocal(out=rec[0:w, :], in_=den[0:w, :])
                    gam = p3_pool.tile([P, 1], f32, name=f"gam_{b}_{o}", tag="gam")
                    nc.vector.tensor_scalar_mul(
                        out=gam[0:w, :], in0=rec[0:w, :], scalar1=0.5
                    )
                    bet = p3_pool.tile([P, 1], f32, name=f"bet_{b}_{o}", tag="bet")
                    nc.vector.tensor_tensor(
                        out=bet[0:w, :], in0=cntc[0:w, :], in1=rec[0:w, :], op=mult
                    )
                    nc.vector.tensor_scalar(
                        out=bet[0:w, :], in0=bet[0:w, :],
                        scalar1=-0.5, scalar2=1.0, op0=mult, op1=add,
                    )
                    o1 = p3_pool.tile([P, d], f32, name=f"o1_{b}_{o}", tag="o1")
                    nc.vector.tensor_scalar_mul(
                        out=o1[0:w, :], in0=ctr_t[0:w, :], scalar1=bet[0:w, :]
                    )
                    ou = p3_pool.tile([P, d], f32, name=f"ou_{b}_{o}", tag="ou")
                    nc.vector.scalar_tensor_tensor(
                        out=ou[0:w, :], in0=trp[0:w, 0:d], scalar=gam[0:w, :],
                        in1=o1[0:w, :], op0=mult, op1=add,
                    )
                    nc.gpsimd.dma_start(
                        out=out[r0:r0 + w, :], in_=ou[0:w, 0:d]
                    )

            # ---------------- phase 1: stream labels ----------------
            for g in range(NG):
                st_psum = psum.tile(
                    [d, GW], f32, name=f"st_psum_{g}", tag="st", bufs=2,
                    space="PSUM",
                )
                # two partial-count accumulators so the elementwise adds
                # split across the DVE (even k-tiles) and GpSimd (odd)
                counts_a = cnts_pool.tile(
                    [P, GW], f32, name=f"counts_a_{g}", tag="cnt_a"
                )
                counts_b = cnts_pool.tile(
                    [P, GW], f32, name=f"counts_b_{g}", tag="cnt_b"
                )
                for t in range(KT):
                    lab_t = lab_pool.tile(
                        [P, GW], f32r, name=f"lab_{g}_{t}", tag="lab"
                    )
                    nc.sync.dma_start(
                        out=lab_t[:],
                        in_=labels[t * P:(t + 1) * P, g * GW:(g + 1) * GW],
                    )
                    for c0, w in _chunks(GW):
                        nc.tensor.matmul(
                            out=st_psum[:, c0:c0 + w],
                            lhsT=preds_sb[:, t * d:(t + 1) * d],
                            rhs=lab_t[:, c0:c0 + w],
                            start=(t == 0),
                            stop=(t == KT - 1),
                        )
                    eng = nc.vector if t % 2 == 0 else nc.gpsimd
                    acc = counts_a if t % 2 == 0 else counts_b
                    if t < 2:
                        eng.tensor_copy(out=acc[:], in_=lab_t[:].bitcast(f32))
                    else:
                        eng.tensor_add(
                            out=acc[:], in0=acc[:], in1=lab_t[:].bitcast(f32)
                        )
                stage_group(g, st_psum, counts_a, counts_b)
                if g == 5:
                    reduce_batch(0)
                elif g == 8:
                    phase3_batch(0)
                elif g == 9:
                    reduce_batch(1)

            # ---------------- tail: remaining updates ----------------
            phase3_batch(1)

    nc.compile()
    return nc


_PROGRAM = None
LAST_RESULTS = None  # BassKernelResults from the most recent run (for test.py)


def _get_program():
    global _PROGRAM
    if _PROGRAM is None:
        _PROGRAM = build_program()
    return _PROGRAM


def kernel(embeded_preds, labels, center):
    from concourse.bass_utils import run_bass_kernel_spmd

    global LAST_RESULTS
    preds = np.ascontiguousarray(np.asarray(embeded_preds, dtype=np.float32))
    lab = np.ascontiguousarray(np.asarray(labels, dtype=np.float32))
    ctr = np.ascontiguousarray(np.asarray(center, dtype=np.float32))
    assert preds.shape == (N, D) and lab.shape == (N, C) and ctr.shape == (C, D)

    nc = _get_program()
    # core i's classes are {g*GW + i*SH + k}: feed it the matching center rows
    ctr_g = ctr.reshape(NG, NCORES, SH, D)
    in_maps = [
        {
            "preds": preds[i * NS:(i + 1) * NS],
            "labels": lab[i * NS:(i + 1) * NS],
            "center": np.ascontiguousarray(ctr_g[:, i].reshape(CS, D)),
        }
        for i in range(NCORES)
    ]
    res = run_bass_kernel_spmd(nc, in_maps, core_ids=list(range(NCORES)))
    LAST_RESULTS = res
    outs = np.stack([res.results[i]["out"] for i in range(NCORES)], axis=0)
    return np.ascontiguousarray(
        outs.reshape(NCORES, NG, SH, D).transpose(1, 0, 2, 3).reshape(C, D)
    )


# revision 13
# speedup vs baseline: 1.1217x; 1.1004x over previous
"""CenterLoss update kernel for Trainium2, 8-core SPMD.

Reference computation (N=16384 samples, C=10000 classes, D=128 dims):
    embeded_labels = labels @ center          # [N,D] gather via one-hot
    diff = embeded_labels - embeded_preds
    grad = (labels.T @ diff) / (counts + 1)   # counts = labels.T @ ones
    out  = center - 0.5 * grad

Because each row of ``labels`` is one-hot, ``labels.T @ labels == diag(counts)``,
so the whole thing collapses to a single pass over ``labels``:

    S      = labels.T @ embeded_preds         # [C,D] per-class sum of preds
    counts = column sums of labels            # [C]
    out    = beta * center + gamma * S
             beta  = 1 - 0.5*counts/(counts+1)
             gamma = 0.5/(counts+1)

The 655MB ``labels`` tensor is streamed through the PE exactly once as the
moving matmul operand (computing S.T = preds.T @ labels tile by tile) in a
single fp32r pass (~1e-4 relative error, far inside the 2e-2 gate), with
per-partition partial counts accumulated on the vector engine and reduced by
a ones matmul at each group boundary.  Work is data-parallel over N across 8
cores.  Classes are processed in 5 column groups of 2000 (wide groups keep
each DMA row chunk at 8KB, which the HBM streams at full rate); each group's
partial (S.T ; counts) block is cast to fp16 and staged to DRAM on the ACT
engine's DMA ring (the small strided writes would head-of-line-block the
label stream on the main ring).  Three ReduceScatters (groups 0-1, 2-3, 4)
run *during* the label streaming, so only the last small collective (~36us:
collectives carry a ~30us fixed cost) is exposed at the tail.  Each
ReduceScatter hands core i columns [i*250, (i+1)*250) of every group in its
batch, so the final elementwise update is local; the host reassembles the
group-interleaved class order.
"""

import numpy as np

N, C, D = 16384, 10000, 128
NCORES = 8
NS = N // NCORES        # 2048 rows per core
CS = C // NCORES        # 1250 classes per core
LR = 0.5
P = 128
KT = NS // P            # 16 k-tiles over this core's rows
NG = 5                  # class-column groups
GW = C // NG            # 2000 columns per group
SH = GW // NCORES       # 250 columns per core per group
# (start group, group count, padded row count) per ReduceScatter batch; the
# pad makes each per-rank shard (rows * nb*SH * 2B) a 32-byte multiple.
BATCHES = [(0, 2, 132), (2, 2, 132), (4, 1, 136)]


def _chunks(width, step=512):
    out = []
    c0 = 0
    while c0 < width:
        out.append((c0, min(step, width - c0)))
        c0 += step
    return out


def build_program(ns=NS, c=C, d=D, ncores=NCORES):
    """Build the SPMD Bass program (identical on every core)."""
    import concourse.bacc as bacc
    import concourse.mybir as mybir
    import concourse.tile as tile
    from concourse.masks import make_identity

    f32 = mybir.dt.float32
    f32r = mybir.dt.float32r
    f16 = mybir.dt.float16
    mult = mybir.AluOpType.mult
    add = mybir.AluOpType.add

    assert ns % P == 0 and c % NG == 0 and GW % ncores == 0

    nc = bacc.Bacc(
        "TRN2",
        target_bir_lowering=False,
        debug=False,
        num_devices=ncores,
    )

    # preds/labels are declared float32r (same bits as the host's fp32) so
    # plain HWDGE DMAs can feed fp32r matmuls at full speed (1 cycle/row vs 4
    # for fp32); skipping the true mantissa rounding costs ~1e-4 relative
    # error, far inside the 2e-2 gate.
    preds = nc.dram_tensor("preds", [ns, d], f32r, kind="ExternalInput").ap()
    labels = nc.dram_tensor("labels", [ns, c], f32r, kind="ExternalInput").ap()
    # center rows arrive pre-permuted to this core's (group, col) order.
    center = nc.dram_tensor("center", [CS, d], f32, kind="ExternalInput").ap()
    out = nc.dram_tensor("out", [CS, d], f32, kind="ExternalOutput").ap()

    # phase-3 tiles: (batch, col offset in batch, width, out row start)
    p3_chunks = []
    for b, (g0, nb, _rp) in enumerate(BATCHES):
        wb = nb * SH
        o = 0
        while o < wb:
            w = min(P, wb - o)
            p3_chunks.append((b, o, w, g0 * SH + o))
            o += w

    batch_of = {}
    for b, (g0, nb, _rp) in enumerate(BATCHES):
        for g in range(g0, g0 + nb):
            batch_of[g] = (b, g - g0)

    with tile.TileContext(nc) as tc:
        with (
            tc.tile_pool(name="const", bufs=1) as const_pool,
            tc.tile_pool(name="dram", bufs=1, space="DRAM") as dram_pool,
            tc.tile_pool(name="lab", bufs=8) as lab_pool,
            tc.tile_pool(name="cnts", bufs=2) as cnts_pool,
            tc.tile_pool(name="stage", bufs=2) as stage_pool,
            tc.tile_pool(name="psum", bufs=1, space="PSUM") as psum,
            tc.tile_pool(name="p3", bufs=2) as p3_pool,
        ):
            identity_h = const_pool.tile([P, P], f16, name="identity_h")
            make_identity(nc, identity_h[:])
            ones_h = const_pool.tile([P, 1], f16, name="ones_h")
            nc.vector.memset(ones_h[:], 1.0)

            # preds for this core, as KT stationary [K=128, M=d] tiles
            # (single reduced-precision fp32r PE pass).
            preds_sb = const_pool.tile([P, KT * d], f32r, name="preds_sb")
            for t in range(KT):
                nc.sync.dma_start(
                    out=preds_sb[:, t * d:(t + 1) * d],
                    in_=preds[t * P:(t + 1) * P, :],
                )

            # staging + reduce buffers per ReduceScatter batch
            partials, reds = [], []
            for b, (g0, nb, rp) in enumerate(BATCHES):
                wb = nb * SH
                partial = dram_pool.tile(
                    [ncores, rp, wb], f16, name=f"partial_{b}"
                )
                red = dram_pool.tile([rp, wb], f16, name=f"red_{b}")
                partials.append(partial)
                reds.append(red)

            def stage_group(g, st_psum, counts_h):
                b, goff = batch_of[g]
                cnt_stage = stage_pool.tile(
                    [1, GW], f16, name=f"cnt_stage_{g}", tag="cnt_stage"
                )
                half = GW // 2
                for h in range(2):
                    # fp16 counts matmul: 1 cycle/row, and counts stay exact
                    # (integers <= 2048 are exact in fp16)
                    cnt_psum = psum.tile(
                        [1, half], f32, name=f"cnt_psum_{g}_{h}", tag="cntp",
                        space="PSUM",
                    )
                    for c0, w in _chunks(half):
                        nc.tensor.matmul(
                            out=cnt_psum[0:1, c0:c0 + w],
                            lhsT=ones_h[:],
                            rhs=counts_h[:, h * half + c0:h * half + c0 + w],
                            start=True,
                            stop=True,
                        )
                    nc.scalar.copy(
                        out=cnt_stage[0:1, h * half:(h + 1) * half],
                        in_=cnt_psum[:],
                    )
                st_stage = stage_pool.tile(
                    [P, GW], f16, name=f"st_stage_{g}", tag="st_stage"
                )
                nc.scalar.copy(out=st_stage[:], in_=st_psum[:])
                off = goff * SH
                # issue staging on the ACT ring: the per-rank slices are
                # 500B-chunk strided writes, and on the label ring their
                # descriptor storm head-of-line-blocks the big streaming reads
                for i in range(ncores):
                    nc.scalar.dma_start(
                        out=partials[b][i, 0:d, off:off + SH],
                        in_=st_stage[:, i * SH:(i + 1) * SH],
                    )
                    nc.scalar.dma_start(
                        out=partials[b][i, d:d + 1, off:off + SH],
                        in_=cnt_stage[0:1, i * SH:(i + 1) * SH],
                    )

            def reduce_batch(b):
                nc.gpsimd.collective_compute(
                    "ReduceScatter",
                    mybir.AluOpType.add,
                    replica_groups=[list(range(ncores))],
                    ins=[partials[b][:].opt()],
                    outs=[reds[b][:].opt()],
                )

            def phase3_batch(b):
                g0, nb, _rp = BATCHES[b]
                wb = nb * SH
                st_sh = p3_pool.tile([P, wb], f16, name=f"st_sh_{b}", tag="st_sh")
                nc.gpsimd.dma_start(out=st_sh[:, 0:wb], in_=reds[b][0:d, :])
                cnt_row = p3_pool.tile(
                    [1, wb], f16, name=f"cnt_row_{b}", tag="cnt_row"
                )
                nc.gpsimd.dma_start(out=cnt_row[:, 0:wb], in_=reds[b][d:d + 1, :])
                for (bb, o, w, r0) in p3_chunks:
                    if bb != b:
                        continue
                    ctr_t = p3_pool.tile([P, d], f32, name=f"ctr_{b}_{o}", tag="ctr")
                    nc.gpsimd.dma_start(out=ctr_t[0:w, :], in_=center[r0:r0 + w, :])
                    trp = psum.tile(
                        [P, d], f16, name=f"trp_{b}_{o}", tag="trp", space="PSUM"
                    )
                    nc.tensor.transpose(
                        out=trp[0:w, 0:d],
                        in_=st_sh[:, o:o + w],
                        identity=identity_h[:, 0:d],
                    )
                    cntc = psum.tile(
                        [P, 1], f16, name=f"cntc_{b}_{o}", tag="cntc", space="PSUM"
                    )
                    nc.tensor.transpose(
                        out=cntc[0:w, 0:1],
                        in_=cnt_row[0:1, o:o + w],
                        identity=identity_h[0:1, 0:1],
                    )
                    den = p3_pool.tile([P, 1], f32, name=f"den_{b}_{o}", tag="den")
                    nc.vector.tensor_scalar_add(
                        out=den[0:w, :], in0=cntc[0:w, :], scalar1=1.0
                    )
                    rec = p3_pool.tile([P, 1], f32, name=f"rec_{b}_{o}", tag="rec")
                    nc.vector.reciprocal(out=rec[0:w, :], in_=den[0:w, :])
                    gam = p3_pool.tile([P, 1], f32, name=f"gam_{b}_{o}", tag="gam")
                    nc.vector.tensor_scalar_mul(
                        out=gam[0:w, :], in0=rec[0:w, :], scalar1=0.5
                    )
                    bet = p3_pool.tile([P, 1], f32, name=f"bet_{b}_{o}", tag="bet")
                    nc.vector.tensor_tensor(
                        out=bet[0:w, :], in0=cntc[0:w, :], in1=rec[0:w, :], op=mult
                    )
                    nc.vector.tensor_scalar(
                        out=bet[0:w, :], in0=bet[0:w, :],
                        scalar1=-0.5, scalar2=1.0, op0=mult, op1=add,
                    )
                    o1 = p3_pool.tile([P, d], f32, name=f"o1_{b}_{o}", tag="o1")
                    nc.vector.tensor_scalar_mul(
                        out=o1[0:w, :], in0=ctr_t[0:w, :], scalar1=bet[0:w, :]
                    )
                    ou = p3_pool.tile([P, d], f32, name=f"ou_{b}_{o}", tag="ou")
                    nc.vector.scalar_tensor_tensor(
                        out=ou[0:w, :], in0=trp[0:w, 0:d], scalar=gam[0:w, :],
                        in1=o1[0:w, :], op0=mult, op1=add,
                    )
                    nc.gpsimd.dma_start(
                        out=out[r0:r0 + w, :], in_=ou[0:w, 0:d]
                    )

            # ---------------- phase 1: stream labels ----------------
            for g in range(NG):
                st_psum = psum.tile(
                    [d, GW], f32, name=f"st_psum_{g}", tag="st", space="PSUM"
                )
                counts_g = cnts_pool.tile(
                    [P, GW], f32, name=f"counts_{g}", tag="cnt_sb"
                )
                for t in range(KT):
                    lab_t = lab_pool.tile(
                        [P, GW], f32r, name=f"lab_{g}_{t}", tag="lab"
                    )
                    nc.sync.dma_start(
                        out=lab_t[:],
                        in_=labels[t * P:(t + 1) * P, g * GW:(g + 1) * GW],
                    )
                    for c0, w in _chunks(GW):
                        nc.tensor.matmul(
                            out=st_psum[:, c0:c0 + w],
                            lhsT=preds_sb[:, t * d:(t + 1) * d],
                            rhs=lab_t[:, c0:c0 + w],
                            start=(t == 0),
                            stop=(t == KT - 1),
                        )
                    if t == 0:
                        nc.vector.tensor_copy(
                            out=counts_g[:], in_=lab_t[:].bitcast(f32)
                        )
                    else:
                        nc.vector.tensor_add(
                            out=counts_g[:],
                            in0=counts_g[:],
                            in1=lab_t[:].bitcast(f32),
                        )
                # fp16 shadow of the counts so the ones-matmul runs at
                # 1 cycle/row instead of fp32's 4
                counts_hh = cnts_pool.tile(
                    [P, GW], f16, name=f"counts_h_{g}", tag="cnt_h"
                )
                nc.vector.tensor_copy(out=counts_hh[:], in_=counts_g[:])
                stage_group(g, st_psum, counts_hh)
                if g == 1:
                    reduce_batch(0)
                elif g == 3:
                    reduce_batch(1)
                    phase3_batch(0)
                elif g == 4:
                    reduce_batch(2)

            # ---------------- tail: remaining updates ----------------
            phase3_batch(1)
            phase3_batch(2)

    nc.compile()
    return nc


_PROGRAM = None
LAST_RESULTS = None  # BassKernelResults from the most recent run (for test.py)


def _get_program():
    global _PROGRAM
    if _PROGRAM is None:
        _PROGRAM = build_program()
    return _PROGRAM


def kernel(embeded_preds, labels, center):
    from concourse.bass_utils import run_bass_kernel_spmd

    global LAST_RESULTS
    preds = np.ascontiguousarray(np.asarray(embeded_preds, dtype=np.float32))
    lab = np.ascontiguousarray(np.asarray(labels, dtype=np.float32))
    ctr = np.ascontiguousarray(np.asarray(center, dtype=np.float32))
    assert preds.shape == (N, D) and lab.shape == (N, C) and ctr.shape == (C, D)

    nc = _get_program()
    # core i's classes are {g*GW + i*SH + k}: feed it the matching center rows
    ctr_g = ctr.reshape(NG, NCORES, SH, D)
    in_maps = [
        {
            "preds": preds[i * NS:(i + 1) * NS],
            "labels": lab[i * NS:(i + 1) * NS],
            "center": np.ascontiguousarray(ctr_g[:, i].reshape(CS, D)),
        }
        for i in range(NCORES)
    ]
    res = run_bass_kernel_spmd(nc, in_maps, core_ids=list(range(NCORES)))
    LAST_RESULTS = res
    outs = np.stack([res.results[i]["out"] for i in range(NCORES)], axis=0)
    return np.ascontiguousarray(
        outs.reshape(NCORES, NG, SH, D).transpose(1, 0, 2, 3).reshape(C, D)
    )


# revision 17
# speedup vs baseline: 1.6492x; 1.4702x over previous
"""CenterLoss update kernel for Trainium2, 8-core SPMD.

Reference computation (N=16384 samples, C=10000 classes, D=128 dims):
    embeded_labels = labels @ center          # [N,D] gather via one-hot
    diff = embeded_labels - embeded_preds
    grad = (labels.T @ diff) / (counts + 1)   # counts = labels.T @ ones
    out  = center - 0.5 * grad

Because each row of ``labels`` is one-hot, ``labels.T @ labels == diag(counts)``,
so the whole thing collapses to a single pass over ``labels``:

    S      = labels.T @ embeded_preds         # [C,D] per-class sum of preds
    counts = column sums of labels            # [C]
    out    = beta * center + gamma * S
             beta  = 1 - 0.5*counts/(counts+1)
             gamma = 0.5/(counts+1)

Sharding: by CLASS, not by batch.  Core i gets the full preds (8MB, a 3%
traffic overhead) plus its own 1250 label *columns* and center rows, and
computes its S shard and counts completely locally -- no collective, no
cross-core reduction, no staging, and no ~30us fixed-cost collective exposed
at the tail (a ReduceScatter variant of this kernel measured 414us against
this design's much shorter critical path).  The host hands each core a
*contiguous* copy of its label column slice, so every k-tile DMA is one
contiguous 640KB read -- the ideal HBM stream pattern.

The 655MB ``labels`` tensor is streamed through the PE exactly once as the
moving matmul operand (computing S.T = preds.T @ labels, accumulating all
128 k-tiles into one PSUM group) in a single fp32r pass (~1e-4 relative
error, far inside the 2e-2 gate).  Per-partition partial counts accumulate
on the vector engine; at the tail they are reduced by a ones matmul (via an
fp16 shadow: 1 PE cycle/row, and class counts here are far below 2048 so
fp16 keeps them exact) and the update is applied per 128-class chunk.
"""

import numpy as np

N, C, D = 16384, 10000, 128
NCORES = 8
CSH = C // NCORES       # 1250 classes per core
LR = 0.5
P = 128
KT = N // P             # 128 k-tiles over the full batch


def _chunks(width, step=512):
    out = []
    c0 = 0
    while c0 < width:
        out.append((c0, min(step, width - c0)))
        c0 += step
    return out


def build_program(n=N, csh=CSH, d=D, ncores=NCORES):
    """Build the SPMD Bass program (identical on every core)."""
    import concourse.bacc as bacc
    import concourse.mybir as mybir
    import concourse.tile as tile
    from concourse.masks import make_identity

    f32 = mybir.dt.float32
    f32r = mybir.dt.float32r
    f16 = mybir.dt.float16
    mult = mybir.AluOpType.mult
    add = mybir.AluOpType.add

    assert n % P == 0

    nc = bacc.Bacc(
        "TRN2",
        target_bir_lowering=False,
        debug=False,
        num_devices=ncores,
    )

    # preds/labels are declared float32r (same bits as the host's fp32) so
    # plain HWDGE DMAs can feed fp32r matmuls at full speed (1 cycle/row vs 4
    # for fp32); skipping the true mantissa rounding costs ~1e-4 relative
    # error, far inside the 2e-2 gate.
    preds = nc.dram_tensor("preds", [n, d], f32r, kind="ExternalInput").ap()
    labels = nc.dram_tensor("labels", [n, csh], f32r, kind="ExternalInput").ap()
    center = nc.dram_tensor("center", [csh, d], f32, kind="ExternalInput").ap()
    out = nc.dram_tensor("out", [csh, d], f32, kind="ExternalOutput").ap()

    with tile.TileContext(nc) as tc:
        with (
            tc.tile_pool(name="const", bufs=1) as const_pool,
            tc.tile_pool(name="lab", bufs=8) as lab_pool,
            tc.tile_pool(name="psum", bufs=1, space="PSUM") as psum,
            tc.tile_pool(name="p3", bufs=2) as p3_pool,
        ):
            identity = const_pool.tile([P, P], f32, name="identity")
            make_identity(nc, identity[:])
            ones_h = const_pool.tile([P, 1], f16, name="ones_h")
            nc.vector.memset(ones_h[:], 1.0)

            # full preds as KT stationary [K=128, M=d] tiles (64KB/partition)
            preds_sb = const_pool.tile([P, KT * d], f32r, name="preds_sb")
            counts_g = const_pool.tile([P, csh], f32, name="counts_g")

            # S.T accumulates across all 128 k-tiles in one PSUM group.
            # Width padded to 3 full PSUM banks: matmul outputs may not cross
            # a bank boundary, and the 226-col remainder chunk would run at
            # 1/4 throughput (fp32r needs >=256 moving columns) -- so run
            # three full 512-wide matmuls instead and let the pad columns
            # compute garbage that is never read.
            cpad = 3 * 512
            st_psum = psum.tile([d, cpad], f32, name="st_psum", tag="st",
                                space="PSUM")

            # ---------------- phase 1: stream labels ----------------
            for t in range(KT):
                # just-in-time preds tile keeps the ring mostly-labels
                nc.sync.dma_start(
                    out=preds_sb[:, t * d:(t + 1) * d],
                    in_=preds[t * P:(t + 1) * P, :],
                )
                lab_t = lab_pool.tile([P, cpad], f32r, name=f"lab_{t}", tag="lab")
                nc.sync.dma_start(
                    out=lab_t[:, 0:csh], in_=labels[t * P:(t + 1) * P, :]
                )
                for c0 in (0, 512, 1024):
                    nc.tensor.matmul(
                        out=st_psum[:, c0:c0 + 512],
                        lhsT=preds_sb[:, t * d:(t + 1) * d],
                        rhs=lab_t[:, c0:c0 + 512],
                        start=(t == 0),
                        stop=(t == KT - 1),
                    )
                if t == 0:
                    nc.vector.tensor_copy(
                        out=counts_g[:], in_=lab_t[:, 0:csh].bitcast(f32)
                    )
                else:
                    nc.vector.tensor_add(
                        out=counts_g[:],
                        in0=counts_g[:],
                        in1=lab_t[:, 0:csh].bitcast(f32),
                    )

            # ---------------- tail: counts + update ----------------
            # fp16 shadow of the counts so the ones-matmul runs at
            # 1 cycle/row instead of fp32's 4
            counts_h = const_pool.tile([P, csh], f16, name="counts_h")
            nc.vector.tensor_copy(out=counts_h[:], in_=counts_g[:])
            cnt_psum = psum.tile([1, csh], f32, name="cnt_psum", tag="cntp",
                                 space="PSUM")
            for c0, w in _chunks(csh):
                nc.tensor.matmul(
                    out=cnt_psum[0:1, c0:c0 + w],
                    lhsT=ones_h[:],
                    rhs=counts_h[:, c0:c0 + w],
                    start=True,
                    stop=True,
                )
            cnt_row = const_pool.tile([1, csh], f32, name="cnt_row")
            nc.scalar.copy(out=cnt_row[:], in_=cnt_psum[:])

            nt3 = (csh + P - 1) // P
            st_sb = const_pool.tile([P, nt3 * P], f32, name="st_sb")
            for tt in range(nt3):
                w = min(P, csh - tt * P)
                # chunked PSUM evacuation so the first transpose starts early
                nc.scalar.copy(
                    out=st_sb[:, tt * P:tt * P + w],
                    in_=st_psum[:, tt * P:tt * P + w],
                )
                ctr_t = p3_pool.tile([P, d], f32, name=f"ctr_{tt}", tag="ctr")
                nc.gpsimd.dma_start(
                    out=ctr_t[0:w, :], in_=center[tt * P:tt * P + w, :]
                )
                trp = psum.tile([P, d], f32, name=f"trp_{tt}", tag="trp",
                                space="PSUM")
                nc.tensor.transpose(
                    out=trp[0:w, 0:d],
                    in_=st_sb[:, tt * P:tt * P + w],
                    identity=identity[:, 0:d],
                )
                cntc = psum.tile([P, 1], f32, name=f"cntc_{tt}", tag="cntc",
                                 space="PSUM")
                nc.tensor.transpose(
                    out=cntc[0:w, 0:1],
                    in_=cnt_row[0:1, tt * P:tt * P + w],
                    identity=identity[0:1, 0:1],
                )
                den = p3_pool.tile([P, 1], f32, name=f"den_{tt}", tag="den")
                nc.vector.tensor_scalar_add(
                    out=den[0:w, :], in0=cntc[0:w, :], scalar1=1.0
                )
                rec = p3_pool.tile([P, 1], f32, name=f"rec_{tt}", tag="rec")
                nc.vector.reciprocal(out=rec[0:w, :], in_=den[0:w, :])
                gam = p3_pool.tile([P, 1], f32, name=f"gam_{tt}", tag="gam")
                nc.vector.tensor_scalar_mul(
                    out=gam[0:w, :], in0=rec[0:w, :], scalar1=0.5
                )
                bet = p3_pool.tile([P, 1], f32, name=f"bet_{tt}", tag="bet")
                nc.vector.tensor_tensor(
                    out=bet[0:w, :], in0=cntc[0:w, :], in1=rec[0:w, :], op=mult
                )
                nc.vector.tensor_scalar(
                    out=bet[0:w, :], in0=bet[0:w, :],
                    scalar1=-0.5, scalar2=1.0, op0=mult, op1=add,
                )
                o1 = p3_pool.tile([P, d], f32, name=f"o1_{tt}", tag="o1")
                nc.vector.tensor_scalar_mul(
                    out=o1[0:w, :], in0=ctr_t[0:w, :], scalar1=bet[0:w, :]
                )
                ou = p3_pool.tile([P, d], f32, name=f"ou_{tt}", tag="ou")
                nc.vector.scalar_tensor_tensor(
                    out=ou[0:w, :], in0=trp[0:w, 0:d], scalar=gam[0:w, :],
                    in1=o1[0:w, :], op0=mult, op1=add,
                )
                nc.gpsimd.dma_start(
                    out=out[tt * P:tt * P + w, :], in_=ou[0:w, 0:d]
                )

    nc.compile()
    return nc


_PROGRAM = None
LAST_RESULTS = None  # BassKernelResults from the most recent run (for test.py)


def _get_program():
    global _PROGRAM
    if _PROGRAM is None:
        _PROGRAM = build_program()
    return _PROGRAM


def kernel(embeded_preds, labels, center):
    from concourse.bass_utils import run_bass_kernel_spmd

    global LAST_RESULTS
    preds = np.ascontiguousarray(np.asarray(embeded_preds, dtype=np.float32))
    lab = np.ascontiguousarray(np.asarray(labels, dtype=np.float32))
    ctr = np.ascontiguousarray(np.asarray(center, dtype=np.float32))
    assert preds.shape == (N, D) and lab.shape == (N, C) and ctr.shape == (C, D)

    nc = _get_program()
    in_maps = [
        {
            "preds": preds,
            "labels": np.ascontiguousarray(lab[:, i * CSH:(i + 1) * CSH]),
            "center": ctr[i * CSH:(i + 1) * CSH],
        }
        for i in range(NCORES)
    ]
    res = run_bass_kernel_spmd(nc, in_maps, core_ids=list(range(NCORES)))
    LAST_RESULTS = res
    return np.concatenate([res.results[i]["out"] for i in range(NCORES)], axis=0)
